# revision 19
# baseline (speedup 1.0000x reference)
"""Trainium2 Bass kernel for nn_ATTenModel_38809324486671.

Model: y = entmax15_straight_through(relu(x@W1.T+b1) @ Wc.T + bc) @ w2.T + b2
with only 2 logits. In the forward pass the straight-through entmax output is
exactly the one-hot argmax of the logits, so

    y[n] = (h[n] . dw + db >= 0) ? (w2[0,0]+b2) : (w2[0,1]+b2),
    h[n] = relu(x[n] @ W1.T + b1),  dw = wc[0]-wc[1], db = bc[0]-bc[1]

Precision: the output is binary per row, so only rows with |logit-diff|
near zero can flip. Measured on the real data, single fp16 x and W give 49
flipped rows out of 524288 (budget at rel=2e-2 is ~2861), so no hi/lo
splitting is needed. |dw_f| is folded into W1 row f (dw_f*relu(h) =
sign(dw_f)*relu(|dw_f|h)), and the per-feature sign goes into the
reduction weights, so per 512-row chunk the whole model is:

  - 2 fp16 matmuls (K=128+72) -> h' in PSUM          [PE]
  - u16 = fp16(relu(h' + b')) one tensor_scalar op   [DVE]
  - 1 fp16 matmul with +-1 stationary (32-wide band
    of a shared PSUM bank; 4 chunks/bank) -> g       [PE]
  - per 4 chunks: sign(g+db), mid+hd*sign, DMA out   [ACT]

Data-parallel over 8 NeuronCores (65536 rows each). x is shipped as a plain
fp16 transpose [200, N] per core; y is written in plain row order (the
32-partition replication makes each chunk's y a contiguous 2KB DMA).
"""

import sys
import types

import numpy as np

# Defensive: if BASS_TRACE is set in the environment, run_bass_kernel_spmd
# imports antenv.axon_hooks, which this image lacks. Provide a no-op shim
# (returns no hook -> tracing is skipped, run proceeds) unless one exists.
try:
    import antenv.axon_hooks  # noqa: F401
except Exception:
    try:
        import antenv
        _hooks_mod = types.ModuleType("antenv.axon_hooks")
        _hooks_mod._hook = None
        _hooks_mod.set_axon_ntff_profile_hook = (
            lambda h: setattr(_hooks_mod, "_hook", h))
        _hooks_mod.get_axon_ntff_profile_hook = lambda: _hooks_mod._hook
        antenv.axon_hooks = _hooks_mod
        sys.modules["antenv.axon_hooks"] = _hooks_mod
    except Exception:
        pass

import concourse.bacc as bacc
import concourse.tile as tile
from concourse import mybir
from concourse import bass_utils

N_CORES = 8
N_TOTAL = 524288
D_IN = 200
D_H = 100
N_SHARD = N_TOTAL // N_CORES          # 65536
CHUNK = 512
N_CHUNKS = N_SHARD // CHUNK           # 128
# Variable DMA-group sizes (in chunks): small at the edges (fast first
# matmul, short drain); 4-chunk groups in steady state (more concurrent
# DMA instructions keep more of the 16 DMA engines busy than 8-chunk ones).
GROUP_SIZES = [2, 2] + [4] * 30 + [2, 2]
assert sum(GROUP_SIZES) == N_CHUNKS
XU_BUFS = 6
KA, KB = 128, D_IN - 128              # K split of the 200-row contraction
F16_SCALE = 64.0                      # power-of-2: dodges fp16 subnormals in W1

TRACE = False                         # test harness sets True for profiling
LAST_RESULT = {}                      # test harness reads exec_time_ns


def _build(hd_scale: float, mid_v: float):
    f32 = mybir.dt.float32
    f16 = mybir.dt.float16
    DH = D_H
    nc = bacc.Bacc("TRN2", target_bir_lowering=False, debug=False,
                   num_devices=N_CORES)

    y = nc.dram_tensor("y", [N_SHARD], f32, kind="ExternalOutput").ap()
    y_r = y.rearrange("(q s m) -> q s m", q=N_CHUNKS // 4, s=4, m=CHUNK)
    xu = nc.dram_tensor("xu", [D_IN, N_SHARD], f16, kind="ExternalInput").ap()
    wa_d = nc.dram_tensor("wa", [KA, DH], f16, kind="ExternalInput").ap()
    wb_d = nc.dram_tensor("wb", [KB, DH], f16, kind="ExternalInput").ap()
    s32_d = nc.dram_tensor("s32", [DH, 32], f16, kind="ExternalInput").ap()
    bias_d = nc.dram_tensor("bias", [DH, 1], f32, kind="ExternalInput").ap()
    db_d = nc.dram_tensor("db128", [128, 1], f32, kind="ExternalInput").ap()

    with tile.TileContext(nc) as tc:
        with (
            tc.tile_pool(name="consts", bufs=1) as consts,
            tc.tile_pool(name="xu_p", bufs=XU_BUFS) as xu_pool,
            tc.tile_pool(name="rh_p", bufs=6) as rh_pool,
            tc.tile_pool(name="fin", bufs=4) as fin_pool,
            tc.tile_pool(name="ps_h", bufs=1, space="PSUM") as psh_pool,
            tc.tile_pool(name="ps_g", bufs=1, space="PSUM") as psg_pool,
        ):
            # consts go out on the GpSimd DGE queue so the first xa/xb DMAs
            # (Sync/Scalar queues) are not stuck behind them at startup.
            wa_t = consts.tile([KA, DH], f16, tag="wa")
            nc.gpsimd.dma_start(wa_t[:], wa_d[:])
            wb_t = consts.tile([KB, DH], f16, tag="wb")
            nc.gpsimd.dma_start(wb_t[:], wb_d[:])
            s32_t = consts.tile([DH, 32], f16, tag="s32")
            nc.gpsimd.dma_start(s32_t[:], s32_d[:])
            b_t = consts.tile([DH, 1], f32, tag="bias")
            nc.gpsimd.dma_start(b_t[:], bias_d[:])
            db_t = consts.tile([128, 1], f32, tag="db")
            nc.gpsimd.dma_start(db_t[:], db_d[:])

            g_ts = [psg_pool.tile([128, CHUNK], f32, tag=f"g{i}", name=f"g{i}_t")
                    for i in range(2)]

            def emit_mm2(items):
                # g = sum_f s_f * u_f: one fp16 matmul with the +-1 sign
                # stationary (32 replicated columns so each chunk owns a
                # 32-partition band of the g bank; the 4 bands use distinct
                # PE column groups).
                for c, u_ap in items:
                    quad, s4 = divmod(c, 4)
                    gq = g_ts[quad % 2]
                    nc.tensor.matmul(gq[32 * s4:32 * s4 + 32, :], s32_t[:],
                                     u_ap, start=True, stop=True,
                                     tile_position=(0, 32 * s4))
                    if s4 == 3:
                        sgn = fin_pool.tile([128, CHUNK], f32, tag="sgn",
                                            name=f"sgn_{quad}")
                        nc.scalar.activation(
                            sgn[:], gq[:],
                            mybir.ActivationFunctionType.Sign,
                            bias=db_t[:, 0:1], scale=1.0)
                        # y = mid + hd*sgn on the Pool engine (SBUF->SBUF;
                        # GPSIMD has no PSUM port, so Sign stays on ACT).
                        y4 = fin_pool.tile([128, CHUNK], f32, tag="y4",
                                           name=f"y4_{quad}")
                        nc.gpsimd.tensor_scalar(
                            y4[:], sgn[:], hd_scale, float(mid_v),
                            mybir.AluOpType.mult, mybir.AluOpType.add)
                        # rows 0/32/64/96 hold chunks 4q..4q+3; each row is
                        # a contiguous 2KB run of y.
                        nc.gpsimd.dma_start(y_r[quad], y4[0:128:32, :])

            pending = []
            gc0 = 0
            pair_idx = 0
            for grp, gs in enumerate(GROUP_SIZES):
                lo, hi = gc0 * CHUNK, (gc0 + gs) * CHUNK
                # Both on the Sync DGE queue: issuing from the Scalar queue
                # measurably delays ACT compute (relu/sign) behind 667ns DMA
                # issues, stalling MM2s; and the DMA-engine striping is fixed
                # per transfer type anyway, so a second queue adds no bw.
                xa = xu_pool.tile([KA, gs * CHUNK], f16, tag=f"xa{gs}",
                                  name=f"xa_{grp}")
                nc.sync.dma_start(xa[:], xu[0:KA, lo:hi])
                xb = xu_pool.tile([KB, gs * CHUNK], f16, tag=f"xb{gs}",
                                  name=f"xb_{grp}")
                nc.sync.dma_start(xb[:], xu[KA:D_IN, lo:hi])
                # chunk-pairs; each pair owns a 2-bank PSUM tile so
                # relu+bias+fp16 runs as ONE [100,1024] op per pair.
                # Process at most 2 pairs per subgroup (3 pair-tags = 6 PSUM
                # banks; >2 pairs in flight would self-collide on tags).
                npr = gs // 2
                for p0 in range(0, npr, 2):
                    prs = range(p0, min(p0 + 2, npr))
                    pp = {pr: psh_pool.tile([DH, 2 * CHUNK], f32,
                                            name=f"ps_{grp}_{pr}",
                                            tag=f"pp{(pair_idx + pr - p0) % 3}")
                          for pr in prs}
                    for wt, xt, kk, st in ((wa_t, xa, KA, True),
                                           (wb_t, xb, KB, False)):
                        for pr in prs:
                            for h in range(2):
                                cc = 2 * pr + h
                                nc.tensor.matmul(
                                    pp[pr][:, h * CHUNK:(h + 1) * CHUNK],
                                    wt[:],
                                    xt[:kk, cc * CHUNK:(cc + 1) * CHUNK],
                                    start=st, stop=not st)
                    # MM2s of the previous subgroup (their relu inputs are
                    # ready) -- keeps the PE queue from stalling.
                    emit_mm2(pending)
                    pending = []
                    for pr in prs:
                        c0 = gc0 + 2 * pr
                        u16 = rh_pool.tile([DH, 2 * CHUNK], f16, tag="u16",
                                           name=f"u16_{pair_idx}")
                        if pair_idx % 3 < 2:
                            nc.vector.tensor_scalar(
                                u16[:], pp[pr][:], b_t[:, 0:1], 0.0,
                                mybir.AluOpType.add, mybir.AluOpType.max)
                        else:
                            nc.scalar.activation(
                                u16[:], pp[pr][:],
                                mybir.ActivationFunctionType.Relu,
                                bias=b_t[:, 0:1], scale=1.0)
                        pending.append((c0, u16[:, 0:CHUNK]))
                        pending.append((c0 + 1, u16[:, CHUNK:2 * CHUNK]))
                        pair_idx += 1
                gc0 += gs
            emit_mm2(pending)
    nc.compile()
    return nc


def _prep(x, w_out, b_out, w_cat, b_cat, w2, b2):
    scale = np.float32(F16_SCALE)
    dw = (w_cat[0] - w_cat[1]).astype(np.float32)             # [100]
    adw = np.abs(dw)
    sgn = np.where(dw >= 0, 1.0, -1.0).astype(np.float32)
    # dw_f*relu(h_f) = sgn_f*relu(|dw_f|*h_f): fold |dw|*scale into W1, b1
    W1f = (np.ascontiguousarray(w_out.T) * adw[None, :] * scale)  # [200, 100]
    bv = (b_out * adw * scale).reshape(D_H, 1).astype(np.float32)
    db = np.float32(b_cat[0] - b_cat[1]) * scale
    v0 = np.float32(w2[0, 0] + b2[0])
    v1 = np.float32(w2[0, 1] + b2[0])
    mid = float((v0.astype(np.float64) + v1) / 2)
    hd = float((v0.astype(np.float64) - v1) / 2)

    Wh = W1f.astype(np.float16)
    base = {
        "wa": np.ascontiguousarray(Wh[0:KA]),
        "wb": np.ascontiguousarray(Wh[KA:D_IN]),
        "s32": np.ascontiguousarray(np.repeat(sgn[:, None], 32, 1)
                                    .astype(np.float16)),
        "bias": bv,
        "db128": np.full((128, 1), db, np.float32),
    }
    xs = x.reshape(N_CORES, N_SHARD, D_IN)
    in_maps = []
    for k in range(N_CORES):
        m = dict(base)
        m["xu"] = np.ascontiguousarray(xs[k].T).astype(np.float16)
        in_maps.append(m)
    return in_maps, hd, mid


def kernel(x, w_out, b_out, w_cat, b_cat, w2, b2):
    x = np.ascontiguousarray(np.asarray(x, dtype=np.float32))
    w_out = np.asarray(w_out, np.float32)
    b_out = np.asarray(b_out, np.float32)
    w_cat = np.asarray(w_cat, np.float32)
    b_cat = np.asarray(b_cat, np.float32)
    w2 = np.asarray(w2, np.float32)
    b2 = np.asarray(b2, np.float32)

    in_maps, hd, mid = _prep(x, w_out, b_out, w_cat, b_cat, w2, b2)
    nc = _build(hd, mid)
    res = bass_utils.run_bass_kernel_spmd(
        nc, in_maps, core_ids=list(range(N_CORES)), trace=TRACE)
    LAST_RESULT["exec_time_ns"] = res.exec_time_ns
    LAST_RESULT["trace"] = (res.instructions_and_trace[1]
                            if res.instructions_and_trace else None)
    out = np.concatenate([np.asarray(res.results[k]["y"]).reshape(N_SHARD)
                          for k in range(N_CORES)])
    return out.reshape(N_TOTAL, 1).astype(np.float32)


# revision 20
# speedup vs baseline: 1.1140x; 1.1140x over previous
"""Trainium2 Bass kernel for nn_ATTenModel_38809324486671.

Model: y = entmax15_straight_through(relu(x@W1.T+b1) @ Wc.T + bc) @ w2.T + b2
with only 2 logits. In the forward pass the straight-through entmax output is
exactly the one-hot argmax of the logits, so

    y[n] = (h[n] . dw + db >= 0) ? (w2[0,0]+b2) : (w2[0,1]+b2),
    h[n] = relu(x[n] @ W1.T + b1),  dw = wc[0]-wc[1], db = bc[0]-bc[1]

Precision: the output is binary per row, so only rows with |logit-diff|
near zero can flip. Measured on the real data, single fp16 x and W give 49
flipped rows out of 524288 (budget at rel=2e-2 is ~2861), so no hi/lo
splitting is needed. |dw_f| is folded into W1 row f (dw_f*relu(h) =
sign(dw_f)*relu(|dw_f|h)), and the per-feature sign goes into the
reduction weights, so per 512-row chunk the whole model is:

  - 2 fp16 matmuls (K=128+72) -> h' in PSUM          [PE]
  - u16 = fp16(relu(h' + b')) one tensor_scalar op   [DVE]
  - 1 fp16 matmul with +-1 stationary (32-wide band
    of a shared PSUM bank; 4 chunks/bank) -> g       [PE]
  - per 4 chunks: sign(g+db), mid+hd*sign, DMA out   [ACT]

Data-parallel over 8 NeuronCores (65536 rows each). x is shipped as a plain
fp16 transpose [200, N] per core; y is written in plain row order (the
32-partition replication makes each chunk's y a contiguous 2KB DMA).
"""

import sys
import types

import numpy as np

# Defensive: if BASS_TRACE is set in the environment, run_bass_kernel_spmd
# imports antenv.axon_hooks, which this image lacks. Provide a no-op shim
# (returns no hook -> tracing is skipped, run proceeds) unless one exists.
try:
    import antenv.axon_hooks  # noqa: F401
except Exception:
    try:
        import antenv
        _hooks_mod = types.ModuleType("antenv.axon_hooks")
        _hooks_mod._hook = None
        _hooks_mod.set_axon_ntff_profile_hook = (
            lambda h: setattr(_hooks_mod, "_hook", h))
        _hooks_mod.get_axon_ntff_profile_hook = lambda: _hooks_mod._hook
        antenv.axon_hooks = _hooks_mod
        sys.modules["antenv.axon_hooks"] = _hooks_mod
    except Exception:
        pass

import concourse.bacc as bacc
import concourse.tile as tile
from concourse import mybir
from concourse import bass_utils

N_CORES = 8
N_TOTAL = 524288
D_IN = 200
D_H = 100
N_SHARD = N_TOTAL // N_CORES          # 65536
CHUNK = 512
N_CHUNKS = N_SHARD // CHUNK           # 128
UGROUP = 4                            # chunks per DMA group & PSUM subgroup
UN_GROUPS = N_CHUNKS // UGROUP        # 32
XU_BUFS = 6
KA, KB = 128, D_IN - 128              # K split of the 200-row contraction
F16_SCALE = 64.0                      # power-of-2: dodges fp16 subnormals in W1

TRACE = False                         # test harness sets True for profiling
LAST_RESULT = {}                      # test harness reads exec_time_ns


def _build(hd_scale: float, mid_v: float):
    f32 = mybir.dt.float32
    f16 = mybir.dt.float16
    DH = D_H
    nc = bacc.Bacc("TRN2", target_bir_lowering=False, debug=False,
                   num_devices=N_CORES)

    y = nc.dram_tensor("y", [N_SHARD], f32, kind="ExternalOutput").ap()
    y_r = y.rearrange("(q s m) -> q s m", q=N_CHUNKS // 4, s=4, m=CHUNK)
    xu = nc.dram_tensor("xu", [D_IN, N_SHARD], f16, kind="ExternalInput").ap()
    wa_d = nc.dram_tensor("wa", [KA, DH], f16, kind="ExternalInput").ap()
    wb_d = nc.dram_tensor("wb", [KB, DH], f16, kind="ExternalInput").ap()
    s32_d = nc.dram_tensor("s32", [DH, 32], f16, kind="ExternalInput").ap()
    bias_d = nc.dram_tensor("bias", [DH, 1], f32, kind="ExternalInput").ap()
    db_d = nc.dram_tensor("db128", [128, 1], f32, kind="ExternalInput").ap()

    with tile.TileContext(nc) as tc:
        with (
            tc.tile_pool(name="consts", bufs=1) as consts,
            tc.tile_pool(name="xu_p", bufs=XU_BUFS) as xu_pool,
            tc.tile_pool(name="rh_p", bufs=12) as rh_pool,
            tc.tile_pool(name="fin", bufs=4) as fin_pool,
            tc.tile_pool(name="ps_h", bufs=1, space="PSUM") as psh_pool,
            tc.tile_pool(name="ps_g", bufs=1, space="PSUM") as psg_pool,
        ):
            wa_t = consts.tile([KA, DH], f16, tag="wa")
            nc.sync.dma_start(wa_t[:], wa_d[:])
            wb_t = consts.tile([KB, DH], f16, tag="wb")
            nc.sync.dma_start(wb_t[:], wb_d[:])
            s32_t = consts.tile([DH, 32], f16, tag="s32")
            nc.sync.dma_start(s32_t[:], s32_d[:])
            b_t = consts.tile([DH, 1], f32, tag="bias")
            nc.sync.dma_start(b_t[:], bias_d[:])
            db_t = consts.tile([128, 1], f32, tag="db")
            nc.sync.dma_start(db_t[:], db_d[:])

            g_ts = [psg_pool.tile([128, CHUNK], f32, tag=f"g{i}", name=f"g{i}_t")
                    for i in range(2)]

            def emit_mm2(items):
                # g = sum_f s_f * u_f: one fp16 matmul with the +-1 sign
                # stationary (32 replicated columns so each chunk owns a
                # 32-partition band of the g bank; the 4 bands use distinct
                # PE column groups).
                for c, u_ap in items:
                    quad, s4 = divmod(c, 4)
                    gq = g_ts[quad % 2]
                    nc.tensor.matmul(gq[32 * s4:32 * s4 + 32, :], s32_t[:],
                                     u_ap, start=True, stop=True,
                                     tile_position=(0, 32 * s4))
                    if s4 == 3:
                        sgn = fin_pool.tile([128, CHUNK], f32, tag="sgn",
                                            name=f"sgn_{quad}")
                        nc.scalar.activation(
                            sgn[:], gq[:],
                            mybir.ActivationFunctionType.Sign,
                            bias=db_t[:, 0:1], scale=1.0)
                        # y = mid + hd*sgn on the Pool engine (SBUF->SBUF;
                        # GPSIMD has no PSUM port, so Sign stays on ACT).
                        y4 = fin_pool.tile([128, CHUNK], f32, tag="y4",
                                           name=f"y4_{quad}")
                        nc.gpsimd.tensor_scalar(
                            y4[:], sgn[:], hd_scale, float(mid_v),
                            mybir.AluOpType.mult, mybir.AluOpType.add)
                        # rows 0/32/64/96 hold chunks 4q..4q+3; each row is
                        # a contiguous 2KB run of y.
                        nc.gpsimd.dma_start(y_r[quad], y4[0:128:32, :])

            pending = []
            for grp in range(UN_GROUPS):
                gc0 = grp * UGROUP
                lo, hi = gc0 * CHUNK, (gc0 + UGROUP) * CHUNK
                xa = xu_pool.tile([KA, UGROUP * CHUNK], f16, tag="xa",
                                  name=f"xa_{grp}")
                nc.sync.dma_start(xa[:], xu[0:KA, lo:hi])
                xb = xu_pool.tile([KB, UGROUP * CHUNK], f16, tag="xb",
                                  name=f"xb_{grp}")
                nc.sync.dma_start(xb[:], xu[KA:D_IN, lo:hi])
                # 2 chunk-pairs per group; each pair owns a 2-bank PSUM tile
                # so relu+bias+fp16 runs as ONE [100,1024] op per pair.
                pp = [psh_pool.tile([DH, 2 * CHUNK], f32,
                                    name=f"ps_{grp}_{pr}",
                                    tag=f"pp{(2 * grp + pr) % 3}")
                      for pr in range(2)]
                for wt, xt, kk, st in ((wa_t, xa, KA, True),
                                       (wb_t, xb, KB, False)):
                    for pr in range(2):
                        for h in range(2):
                            cc = 2 * pr + h
                            nc.tensor.matmul(
                                pp[pr][:, h * CHUNK:(h + 1) * CHUNK], wt[:],
                                xt[:kk, cc * CHUNK:(cc + 1) * CHUNK],
                                start=st, stop=not st)
                # MM2s of the previous group (their relu inputs are ready)
                # -- keeps the PE queue from stalling.
                emit_mm2(pending)
                pending = []
                for pr in range(2):
                    c0 = gc0 + 2 * pr
                    pair_idx = 2 * grp + pr
                    u16 = rh_pool.tile([DH, 2 * CHUNK], f16, tag="u16",
                                       name=f"u16_{pair_idx}")
                    if pair_idx % 3 < 2:
                        nc.vector.tensor_scalar(
                            u16[:], pp[pr][:], b_t[:, 0:1], 0.0,
                            mybir.AluOpType.add, mybir.AluOpType.max)
                    else:
                        nc.scalar.activation(
                            u16[:], pp[pr][:],
                            mybir.ActivationFunctionType.Relu,
                            bias=b_t[:, 0:1], scale=1.0)
                    pending.append((c0, u16[:, 0:CHUNK]))
                    pending.append((c0 + 1, u16[:, CHUNK:2 * CHUNK]))
            emit_mm2(pending)
    nc.compile()
    return nc


def _prep(x, w_out, b_out, w_cat, b_cat, w2, b2):
    scale = np.float32(F16_SCALE)
    dw = (w_cat[0] - w_cat[1]).astype(np.float32)             # [100]
    adw = np.abs(dw)
    sgn = np.where(dw >= 0, 1.0, -1.0).astype(np.float32)
    # dw_f*relu(h_f) = sgn_f*relu(|dw_f|*h_f): fold |dw|*scale into W1, b1
    W1f = (np.ascontiguousarray(w_out.T) * adw[None, :] * scale)  # [200, 100]
    bv = (b_out * adw * scale).reshape(D_H, 1).astype(np.float32)
    db = np.float32(b_cat[0] - b_cat[1]) * scale
    v0 = np.float32(w2[0, 0] + b2[0])
    v1 = np.float32(w2[0, 1] + b2[0])
    mid = float((v0.astype(np.float64) + v1) / 2)
    hd = float((v0.astype(np.float64) - v1) / 2)

    Wh = W1f.astype(np.float16)
    base = {
        "wa": np.ascontiguousarray(Wh[0:KA]),
        "wb": np.ascontiguousarray(Wh[KA:D_IN]),
        "s32": np.ascontiguousarray(np.repeat(sgn[:, None], 32, 1)
                                    .astype(np.float16)),
        "bias": bv,
        "db128": np.full((128, 1), db, np.float32),
    }
    xs = x.reshape(N_CORES, N_SHARD, D_IN)
    in_maps = []
    for k in range(N_CORES):
        m = dict(base)
        m["xu"] = np.ascontiguousarray(xs[k].T).astype(np.float16)
        in_maps.append(m)
    return in_maps, hd, mid


def kernel(x, w_out, b_out, w_cat, b_cat, w2, b2):
    x = np.ascontiguousarray(np.asarray(x, dtype=np.float32))
    w_out = np.asarray(w_out, np.float32)
    b_out = np.asarray(b_out, np.float32)
    w_cat = np.asarray(w_cat, np.float32)
    b_cat = np.asarray(b_cat, np.float32)
    w2 = np.asarray(w2, np.float32)
    b2 = np.asarray(b2, np.float32)

    in_maps, hd, mid = _prep(x, w_out, b_out, w_cat, b_cat, w2, b2)
    nc = _build(hd, mid)
    res = bass_utils.run_bass_kernel_spmd(
        nc, in_maps, core_ids=list(range(N_CORES)), trace=TRACE)
    LAST_RESULT["exec_time_ns"] = res.exec_time_ns
    LAST_RESULT["trace"] = (res.instructions_and_trace[1]
                            if res.instructions_and_trace else None)
    out = np.concatenate([np.asarray(res.results[k]["y"]).reshape(N_SHARD)
                          for k in range(N_CORES)])
    return out.reshape(N_TOTAL, 1).astype(np.float32)
